# revision 25
# baseline (speedup 1.0000x reference)
"""Trainium2 Bass kernel for coverage-attention (sparse_attention nn_Attention).

Computes, for inputs query(B,1,H) states(B,S,H) states_features(B,S,H)
coverage(B,S,1) source_mask(B,S) and params Wq(H,H) bq(H) Wcov(H,1) v(H):
    qf    = query @ Wq.T + bq
    total = qf + states_features + coverage * Wcov[:,0]
    align = tanh(total) @ v
    a     = softmax(align, axis=1)          (source_mask is all-True)
    attn_h = a @ states
    new_coverage = coverage + a[..., None]
    align_vectors = a[..., None]

Sharding: pure data-parallel over batch, B=32 -> 4 batches on each of 8 cores.
"""

import sys

import numpy as np

if "/opt/trn_rl_repo" not in sys.path:
    sys.path.insert(0, "/opt/trn_rl_repo")

B, S, H = 32, 2048, 1024
NCORES = 8
BPC = B // NCORES  # batches per core
NCH = S // 128     # s-chunks per batch (16)
HC = H // 128      # h-chunks (8)

_CACHE = {}

# Which chunks compute the M-term (qf + cov*Wcov) on DVE instead of PE.
# Every DVE_M_EVERY-th chunk uses the DVE path; 0 disables (all PE).
import os as _os

DVE_M_EVERY = int(_os.environ.get("DVE_M_EVERY", "2"))


def _bcast_p(ap, p):
    """AP broadcast across p partitions (partition step 0)."""
    import concourse.bass as bass

    return bass.AP(tensor=ap.tensor, offset=ap.offset, ap=[[0, p]] + list(ap.ap))


def _build_nc(repeat=1):
    from contextlib import ExitStack

    import concourse.bacc as bacc
    import concourse.bass as bass
    import concourse.tile as tile
    from concourse import mybir
    from concourse.masks import make_identity

    f32 = mybir.dt.float32
    AF = mybir.ActivationFunctionType
    ALU = mybir.AluOpType

    nc = bacc.Bacc(
        "TRN2", target_bir_lowering=False, debug=False, num_devices=NCORES
    )

    q_d = nc.dram_tensor("query", (BPC, 1, H), f32, kind="ExternalInput").ap()
    st_d = nc.dram_tensor("states", (BPC, S, H), f32, kind="ExternalInput").ap()
    sf_d = nc.dram_tensor("sfeat", (BPC, S, H), f32, kind="ExternalInput").ap()
    cov_d = nc.dram_tensor("cov", (BPC, S, 1), f32, kind="ExternalInput").ap()
    wq_d = nc.dram_tensor("Wq", (H, H), f32, kind="ExternalInput").ap()
    bq_d = nc.dram_tensor("bq", (H,), f32, kind="ExternalInput").ap()
    wcov_d = nc.dram_tensor("Wcov", (H, 1), f32, kind="ExternalInput").ap()
    v_d = nc.dram_tensor("v", (H,), f32, kind="ExternalInput").ap()

    attn_d = nc.dram_tensor("attn", (BPC, 1, H), f32, kind="ExternalOutput").ap()
    ncov_d = nc.dram_tensor("ncov", (BPC, S, 1), f32, kind="ExternalOutput").ap()
    av_d = nc.dram_tensor("av", (BPC, S, 1), f32, kind="ExternalOutput").ap()

    with tile.TileContext(nc) as tc:
      for _rep in range(repeat):
       with ExitStack() as ctx:
        singles = ctx.enter_context(tc.tile_pool(name="singles", bufs=1))
        sf_pool = ctx.enter_context(tc.tile_pool(name="sf", bufs=4))
        st_pool = ctx.enter_context(tc.tile_pool(name="st", bufs=4))
        t_pool = ctx.enter_context(tc.tile_pool(name="t", bufs=3))
        th_pool = ctx.enter_context(tc.tile_pool(name="th", bufs=3))
        pr_pool = ctx.enter_context(tc.tile_pool(name="pr", bufs=2))
        perb = ctx.enter_context(tc.tile_pool(name="perb", bufs=2))
        small = ctx.enter_context(tc.tile_pool(name="small", bufs=2))
        mps = ctx.enter_context(tc.tile_pool(name="mps", bufs=2, space="PSUM"))
        aps_ = ctx.enter_context(tc.tile_pool(name="aps", bufs=1, space="PSUM"))
        sps = ctx.enter_context(tc.tile_pool(name="sps", bufs=1, space="PSUM"))

        # ---- one-time setup ----
        # v replicated across partitions: [128, H]
        v_rep = singles.tile([128, H], f32)
        nc.sync.dma_start(out=v_rep, in_=_bcast_p(v_d, 128))

        # Wcov row replicated across partitions (for the DVE M-path)
        w_rep = singles.tile([128, H], f32)
        nc.sync.dma_start(out=w_rep, in_=_bcast_p(wcov_d[:, 0], 128))

        dram = ctx.enter_context(tc.tile_pool(name="dram", bufs=1, space="DRAM"))
        qf_scr = dram.tile([BPC, H], f32)

        ident = singles.tile([128, 128], f32)
        make_identity(nc, ident)

        ones128 = singles.tile([128, NCH], f32)
        nc.vector.memset(ones128, 1.0)

        # Wq transposed tiles: wqt[c][p, o] = Wq[o, c*128+p].
        # Load Wq natively (contiguous DMA) and transpose 128x128 blocks on PE;
        # a direct transposed DMA load is ~40x slower (4-byte descriptors).
        wqt = [singles.tile([128, H], f32, tag=f"wqt{c}", name=f"wqt{c}") for c in range(HC)]
        for oc in range(HC):
            wqn = t_pool.tile([128, H], f32, tag="wqn")
            nc.sync.dma_start(out=wqn, in_=wq_d[oc * 128 : (oc + 1) * 128, :])
            for c in range(HC):
                wtp = mps.tile([128, 128], f32, tag="m_psum", name="wtp")
                nc.tensor.transpose(wtp, wqn[:, c * 128 : (c + 1) * 128], ident)
                nc.scalar.copy(
                    out=wqt[c][:, oc * 128 : (oc + 1) * 128], in_=wtp
                )

        # qT[p, c, b] = query[b, 0, c*128+p]
        qT = singles.tile([128, HC, BPC], f32)
        for c in range(HC):
            nc.sync.dma_start(
                out=qT[:, c, :],
                in_=q_d[:, 0, c * 128 : (c + 1) * 128].rearrange("b p -> p b"),
            )

        # qf[b, o] = sum_h q[b,h] Wq[o,h]  (+ bq)
        qf_psum = mps.tile([128, H], f32, tag="m_psum")  # only rows 0..BPC used
        for c in range(HC):
            for n in range(2):
                nc.tensor.matmul(
                    qf_psum[:BPC, bass.ts(n, 512)],
                    lhsT=qT[:, c, :],
                    rhs=wqt[c][:, bass.ts(n, 512)],
                    start=(c == 0),
                    stop=(c == HC - 1),
                )
        bq4 = small.tile([BPC, H], f32, tag="bq4")
        nc.sync.dma_start(out=bq4, in_=_bcast_p(bq_d, BPC))
        qf_sb = singles.tile([BPC, H], f32)
        nc.vector.tensor_tensor(
            out=qf_sb, in0=qf_psum[:BPC, :], in1=bq4, op=ALU.add
        )
        # round-trip through DRAM so qf can be partition-broadcast per batch
        nc.sync.dma_start(out=qf_scr, in_=qf_sb)

        # ---- per-batch streaming ----
        for b in range(BPC):
            # rhs2: row0 = qf[b,:], row1 = Wcov[:,0]
            rhs2 = perb.tile([2, H], f32, tag="rhs2")
            nc.sync.dma_start(out=rhs2[0:1, :], in_=qf_sb[b : b + 1, :])
            nc.sync.dma_start(out=rhs2[1:2, :], in_=wcov_d[:, 0][None, :])

            # lhs2: row0 = ones, row1 = coverage[b,:,0]
            lhs2 = perb.tile([2, S], f32, tag="lhs2")
            nc.vector.memset(lhs2[0:1, :], 1.0)
            nc.sync.dma_start(out=lhs2[1:2, :], in_=cov_d[b, :, 0][None, :])

            # qf[b] replicated across 128 partitions (for the DVE M-path)
            qf_rep = perb.tile([128, H], f32, tag="qf_rep")
            nc.sync.dma_start(out=qf_rep, in_=_bcast_p(qf_scr[b, :], 128))

            e16 = perb.tile([128, NCH], f32, tag="e16")
            align16 = perb.tile([128, NCH], f32, tag="al16")
            attn_psum = aps_.tile([1, H], f32)

            for c in range(NCH):
                sf_t = sf_pool.tile([128, H], f32)
                nc.sync.dma_start(out=sf_t, in_=sf_d[b, bass.ts(c, 128), :])
                st_t = st_pool.tile([128, H], f32)
                nc.sync.dma_start(out=st_t, in_=st_d[b, bass.ts(c, 128), :])

                t_t = t_pool.tile([128, H], f32)
                if DVE_M_EVERY == 0 or c % DVE_M_EVERY != DVE_M_EVERY - 1:
                    # PE path: M[s,h] = qf[h] + cov[s]*Wcov[h] (rank-2, K=2)
                    m_psum = mps.tile([128, H], f32)
                    for n in range(2):
                        nc.tensor.matmul(
                            m_psum[:, bass.ts(n, 512)],
                            lhsT=lhs2[:, bass.ts(c, 128)],
                            rhs=rhs2[:, bass.ts(n, 512)],
                            start=True,
                            stop=True,
                        )
                    nc.vector.tensor_tensor(
                        out=t_t, in0=sf_t, in1=m_psum, op=ALU.add
                    )
                else:
                    # DVE path: (Wcov_rep * cov[s]) + sf, then + qf_rep
                    covc = small.tile([128, 1], f32, tag="covc")
                    nc.sync.dma_start(
                        out=covc, in_=cov_d[b, bass.ts(c, 128), :]
                    )
                    msf = pr_pool.tile([128, H], f32, tag="msf")
                    nc.vector.scalar_tensor_tensor(
                        out=msf,
                        in0=w_rep,
                        scalar=covc,
                        in1=sf_t,
                        op0=ALU.mult,
                        op1=ALU.add,
                    )
                    nc.vector.tensor_tensor(
                        out=t_t, in0=msf, in1=qf_rep, op=ALU.add
                    )
                th_t = th_pool.tile([128, H], f32)
                nc.scalar.activation(out=th_t, in_=t_t, func=AF.Tanh)
                prod = pr_pool.tile([128, H], f32)
                nc.vector.scalar_tensor_tensor(
                    out=prod,
                    in0=th_t,
                    scalar=1.0,
                    in1=v_rep,
                    op0=ALU.mult,
                    op1=ALU.mult,
                    accum_out=align16[:, c : c + 1],
                )
                nc.scalar.activation(
                    out=e16[:, c : c + 1],
                    in_=align16[:, c : c + 1],
                    func=AF.Exp,
                )
                for n in range(2):
                    nc.tensor.matmul(
                        attn_psum[:, bass.ts(n, 512)],
                        lhsT=e16[:, c : c + 1],
                        rhs=st_t[:, bass.ts(n, 512)],
                        start=(c == 0),
                        stop=(c == NCH - 1),
                    )

            # Z replicated on 16 partitions: z16[m, c] = sum_p e16[p, c] for all m
            z_psum = sps.tile([NCH, NCH], f32, tag="z")
            nc.tensor.matmul(z_psum, lhsT=ones128, rhs=e16, start=True, stop=True)
            r16 = small.tile([NCH, 1], f32, tag="r16")
            zsum16 = small.tile([NCH, 1], f32, tag="zsum16")
            nc.vector.tensor_reduce(
                out=zsum16, in_=z_psum, axis=mybir.AxisListType.X, op=ALU.add
            )
            nc.vector.reciprocal(out=r16, in_=zsum16)

            # attn_h[b] = r * attn_psum
            attn_row = small.tile([1, H], f32, tag="attn_row")
            nc.vector.tensor_scalar_mul(attn_row, attn_psum, r16[0:1, :])
            nc.sync.dma_start(out=attn_d[b], in_=attn_row)

            # a (normalized weights), transposed to [16, 128] for contiguous output
            eT_psum = sps.tile([NCH, 128], f32, tag="eT")
            nc.tensor.transpose(eT_psum, e16, ident)
            a_sb = small.tile([NCH, 128], f32, tag="a_sb")
            nc.vector.tensor_scalar_mul(a_sb, eT_psum, r16)
            nc.sync.dma_start(
                out=av_d[b, :, 0].rearrange("(c s) -> c s", c=NCH), in_=a_sb
            )

            # new_coverage = coverage + a
            covT = small.tile([NCH, 128], f32, tag="covT")
            nc.sync.dma_start(
                out=covT, in_=cov_d[b, :, 0].rearrange("(c s) -> c s", c=NCH)
            )
            ncov_sb = small.tile([NCH, 128], f32, tag="ncov_sb")
            nc.vector.tensor_tensor(out=ncov_sb, in0=covT, in1=a_sb, op=ALU.add)
            nc.sync.dma_start(
                out=ncov_d[b, :, 0].rearrange("(c s) -> c s", c=NCH), in_=ncov_sb
            )

    nc.compile()
    return nc


def _get_nc():
    if "nc" not in _CACHE:
        _CACHE["nc"] = _build_nc()
    return _CACHE["nc"]


def _make_in_maps(inputs):
    query = inputs["query"]
    states = inputs["states"]
    states_features = inputs["states_features"]
    coverage = inputs["coverage"]
    Wq = np.ascontiguousarray(inputs["Wq"], dtype=np.float32)
    bq = np.ascontiguousarray(inputs["bq"], dtype=np.float32)
    Wcov = np.ascontiguousarray(inputs["Wcov"], dtype=np.float32)
    v = np.ascontiguousarray(inputs["v"], dtype=np.float32)

    in_maps = []
    for i in range(NCORES):
        sl = slice(i * BPC, (i + 1) * BPC)
        in_maps.append(
            {
                "query": np.ascontiguousarray(query[sl], dtype=np.float32),
                "states": np.ascontiguousarray(states[sl], dtype=np.float32),
                "sfeat": np.ascontiguousarray(states_features[sl], dtype=np.float32),
                "cov": np.ascontiguousarray(coverage[sl], dtype=np.float32),
                "Wq": Wq,
                "bq": bq,
                "Wcov": Wcov,
                "v": v,
            }
        )
    return in_maps


def kernel(query, states, states_features, coverage, source_mask, Wq, bq, Wcov, v):
    from concourse.bass_utils import run_bass_kernel_spmd

    nc = _get_nc()
    in_maps = _make_in_maps(
        {
            "query": query,
            "states": states,
            "states_features": states_features,
            "coverage": coverage,
            "Wq": Wq,
            "bq": bq,
            "Wcov": Wcov,
            "v": v,
        }
    )
    res = run_bass_kernel_spmd(nc, in_maps, core_ids=list(range(NCORES)))
    attn = np.concatenate([r["attn"] for r in res.results], axis=0)
    ncov = np.concatenate([r["ncov"] for r in res.results], axis=0)
    av = np.concatenate([r["av"] for r in res.results], axis=0)
    return attn, ncov, av


# revision 26
# speedup vs baseline: 1.1203x; 1.1203x over previous
"""Trainium2 Bass kernel for coverage-attention (sparse_attention nn_Attention).

Computes, for inputs query(B,1,H) states(B,S,H) states_features(B,S,H)
coverage(B,S,1) source_mask(B,S) and params Wq(H,H) bq(H) Wcov(H,1) v(H):
    qf    = query @ Wq.T + bq
    total = qf + states_features + coverage * Wcov[:,0]
    align = tanh(total) @ v
    a     = softmax(align, axis=1)          (source_mask is all-True)
    attn_h = a @ states
    new_coverage = coverage + a[..., None]
    align_vectors = a[..., None]

Sharding: pure data-parallel over batch, B=32 -> 4 batches on each of 8 cores.
"""

import sys

import numpy as np

if "/opt/trn_rl_repo" not in sys.path:
    sys.path.insert(0, "/opt/trn_rl_repo")

B, S, H = 32, 2048, 1024
NCORES = 8
BPC = B // NCORES  # batches per core
NCH = S // 128     # s-chunks per batch (16)
HC = H // 128      # h-chunks (8)

_CACHE = {}

# Which chunks compute the M-term (qf + cov*Wcov) on DVE instead of PE.
# Every DVE_M_EVERY-th chunk uses the DVE path; balances PE vs DVE load
# (HW-measured: 191us vs 224us for the all-PE variant).
DVE_M_EVERY = 2


def _bcast_p(ap, p):
    """AP broadcast across p partitions (partition step 0)."""
    import concourse.bass as bass

    return bass.AP(tensor=ap.tensor, offset=ap.offset, ap=[[0, p]] + list(ap.ap))


def _build_nc(repeat=1):
    from contextlib import ExitStack

    import concourse.bacc as bacc
    import concourse.bass as bass
    import concourse.tile as tile
    from concourse import mybir
    from concourse.masks import make_identity

    f32 = mybir.dt.float32
    AF = mybir.ActivationFunctionType
    ALU = mybir.AluOpType

    nc = bacc.Bacc(
        "TRN2", target_bir_lowering=False, debug=False, num_devices=NCORES
    )

    q_d = nc.dram_tensor("query", (BPC, 1, H), f32, kind="ExternalInput").ap()
    st_d = nc.dram_tensor("states", (BPC, S, H), f32, kind="ExternalInput").ap()
    sf_d = nc.dram_tensor("sfeat", (BPC, S, H), f32, kind="ExternalInput").ap()
    cov_d = nc.dram_tensor("cov", (BPC, S, 1), f32, kind="ExternalInput").ap()
    wq_d = nc.dram_tensor("Wq", (H, H), f32, kind="ExternalInput").ap()
    bq_d = nc.dram_tensor("bq", (H,), f32, kind="ExternalInput").ap()
    wcov_d = nc.dram_tensor("Wcov", (H, 1), f32, kind="ExternalInput").ap()
    v_d = nc.dram_tensor("v", (H,), f32, kind="ExternalInput").ap()

    attn_d = nc.dram_tensor("attn", (BPC, 1, H), f32, kind="ExternalOutput").ap()
    ncov_d = nc.dram_tensor("ncov", (BPC, S, 1), f32, kind="ExternalOutput").ap()
    av_d = nc.dram_tensor("av", (BPC, S, 1), f32, kind="ExternalOutput").ap()

    with tile.TileContext(nc) as tc:
      for _rep in range(repeat):
       with ExitStack() as ctx:
        singles = ctx.enter_context(tc.tile_pool(name="singles", bufs=1))
        sf_pool = ctx.enter_context(tc.tile_pool(name="sf", bufs=4))
        st_pool = ctx.enter_context(tc.tile_pool(name="st", bufs=4))
        t_pool = ctx.enter_context(tc.tile_pool(name="t", bufs=3))
        th_pool = ctx.enter_context(tc.tile_pool(name="th", bufs=3))
        pr_pool = ctx.enter_context(tc.tile_pool(name="pr", bufs=2))
        perb = ctx.enter_context(tc.tile_pool(name="perb", bufs=2))
        small = ctx.enter_context(tc.tile_pool(name="small", bufs=2))
        mps = ctx.enter_context(tc.tile_pool(name="mps", bufs=2, space="PSUM"))
        aps_ = ctx.enter_context(tc.tile_pool(name="aps", bufs=1, space="PSUM"))
        sps = ctx.enter_context(tc.tile_pool(name="sps", bufs=1, space="PSUM"))

        # ---- one-time setup ----
        # v replicated across partitions: [128, H]
        v_rep = singles.tile([128, H], f32)
        nc.sync.dma_start(out=v_rep, in_=_bcast_p(v_d, 128))

        # Wcov row replicated across partitions (for the DVE M-path)
        w_rep = singles.tile([128, H], f32)
        nc.sync.dma_start(out=w_rep, in_=_bcast_p(wcov_d[:, 0], 128))

        dram = ctx.enter_context(tc.tile_pool(name="dram", bufs=1, space="DRAM"))
        qf_scr = dram.tile([BPC, H], f32)

        ident = singles.tile([128, 128], f32)
        make_identity(nc, ident)

        ones128 = singles.tile([128, NCH], f32)
        nc.vector.memset(ones128, 1.0)

        # Wq transposed tiles: wqt[c][p, o] = Wq[o, c*128+p].
        # Load Wq natively (contiguous DMA) and transpose 128x128 blocks on PE;
        # a direct transposed DMA load is ~40x slower (4-byte descriptors).
        wqt = [singles.tile([128, H], f32, tag=f"wqt{c}", name=f"wqt{c}") for c in range(HC)]
        for oc in range(HC):
            wqn = t_pool.tile([128, H], f32, tag="wqn")
            nc.sync.dma_start(out=wqn, in_=wq_d[oc * 128 : (oc + 1) * 128, :])
            for c in range(HC):
                wtp = mps.tile([128, 128], f32, tag="m_psum", name="wtp")
                nc.tensor.transpose(wtp, wqn[:, c * 128 : (c + 1) * 128], ident)
                nc.scalar.copy(
                    out=wqt[c][:, oc * 128 : (oc + 1) * 128], in_=wtp
                )

        # qT[p, c, b] = query[b, 0, c*128+p]
        qT = singles.tile([128, HC, BPC], f32)
        for c in range(HC):
            nc.sync.dma_start(
                out=qT[:, c, :],
                in_=q_d[:, 0, c * 128 : (c + 1) * 128].rearrange("b p -> p b"),
            )

        # qf[b, o] = sum_h q[b,h] Wq[o,h]  (+ bq)
        qf_psum = mps.tile([128, H], f32, tag="m_psum")  # only rows 0..BPC used
        for c in range(HC):
            for n in range(2):
                nc.tensor.matmul(
                    qf_psum[:BPC, bass.ts(n, 512)],
                    lhsT=qT[:, c, :],
                    rhs=wqt[c][:, bass.ts(n, 512)],
                    start=(c == 0),
                    stop=(c == HC - 1),
                )
        bq4 = small.tile([BPC, H], f32, tag="bq4")
        nc.sync.dma_start(out=bq4, in_=_bcast_p(bq_d, BPC))
        qf_sb = singles.tile([BPC, H], f32)
        nc.vector.tensor_tensor(
            out=qf_sb, in0=qf_psum[:BPC, :], in1=bq4, op=ALU.add
        )
        # round-trip through DRAM so qf can be partition-broadcast per batch
        nc.sync.dma_start(out=qf_scr, in_=qf_sb)

        # ---- per-batch streaming ----
        for b in range(BPC):
            # rhs2: row0 = qf[b,:], row1 = Wcov[:,0]
            rhs2 = perb.tile([2, H], f32, tag="rhs2")
            nc.sync.dma_start(out=rhs2[0:1, :], in_=qf_sb[b : b + 1, :])
            nc.sync.dma_start(out=rhs2[1:2, :], in_=wcov_d[:, 0][None, :])

            # lhs2: row0 = ones, row1 = coverage[b,:,0]
            lhs2 = perb.tile([2, S], f32, tag="lhs2")
            nc.vector.memset(lhs2[0:1, :], 1.0)
            nc.sync.dma_start(out=lhs2[1:2, :], in_=cov_d[b, :, 0][None, :])

            # qf[b] replicated across 128 partitions (for the DVE M-path)
            qf_rep = perb.tile([128, H], f32, tag="qf_rep")
            nc.sync.dma_start(out=qf_rep, in_=_bcast_p(qf_scr[b, :], 128))

            e16 = perb.tile([128, NCH], f32, tag="e16")
            align16 = perb.tile([128, NCH], f32, tag="al16")
            attn_psum = aps_.tile([1, H], f32)

            for c in range(NCH):
                sf_t = sf_pool.tile([128, H], f32)
                nc.sync.dma_start(out=sf_t, in_=sf_d[b, bass.ts(c, 128), :])
                st_t = st_pool.tile([128, H], f32)
                nc.sync.dma_start(out=st_t, in_=st_d[b, bass.ts(c, 128), :])

                t_t = t_pool.tile([128, H], f32)
                if DVE_M_EVERY == 0 or c % DVE_M_EVERY != DVE_M_EVERY - 1:
                    # PE path: M[s,h] = qf[h] + cov[s]*Wcov[h] (rank-2, K=2)
                    m_psum = mps.tile([128, H], f32)
                    for n in range(2):
                        nc.tensor.matmul(
                            m_psum[:, bass.ts(n, 512)],
                            lhsT=lhs2[:, bass.ts(c, 128)],
                            rhs=rhs2[:, bass.ts(n, 512)],
                            start=True,
                            stop=True,
                        )
                    nc.vector.tensor_tensor(
                        out=t_t, in0=sf_t, in1=m_psum, op=ALU.add
                    )
                else:
                    # DVE path: (Wcov_rep * cov[s]) + sf, then + qf_rep
                    covc = small.tile([128, 1], f32, tag="covc")
                    nc.sync.dma_start(
                        out=covc, in_=cov_d[b, bass.ts(c, 128), :]
                    )
                    msf = pr_pool.tile([128, H], f32, tag="msf")
                    nc.vector.scalar_tensor_tensor(
                        out=msf,
                        in0=w_rep,
                        scalar=covc,
                        in1=sf_t,
                        op0=ALU.mult,
                        op1=ALU.add,
                    )
                    nc.vector.tensor_tensor(
                        out=t_t, in0=msf, in1=qf_rep, op=ALU.add
                    )
                th_t = th_pool.tile([128, H], f32)
                nc.scalar.activation(out=th_t, in_=t_t, func=AF.Tanh)
                prod = pr_pool.tile([128, H], f32)
                nc.vector.scalar_tensor_tensor(
                    out=prod,
                    in0=th_t,
                    scalar=1.0,
                    in1=v_rep,
                    op0=ALU.mult,
                    op1=ALU.mult,
                    accum_out=align16[:, c : c + 1],
                )
                nc.scalar.activation(
                    out=e16[:, c : c + 1],
                    in_=align16[:, c : c + 1],
                    func=AF.Exp,
                )
                for n in range(2):
                    nc.tensor.matmul(
                        attn_psum[:, bass.ts(n, 512)],
                        lhsT=e16[:, c : c + 1],
                        rhs=st_t[:, bass.ts(n, 512)],
                        start=(c == 0),
                        stop=(c == NCH - 1),
                    )

            # Z replicated on 16 partitions: z16[m, c] = sum_p e16[p, c] for all m
            z_psum = sps.tile([NCH, NCH], f32, tag="z")
            nc.tensor.matmul(z_psum, lhsT=ones128, rhs=e16, start=True, stop=True)
            r16 = small.tile([NCH, 1], f32, tag="r16")
            zsum16 = small.tile([NCH, 1], f32, tag="zsum16")
            nc.vector.tensor_reduce(
                out=zsum16, in_=z_psum, axis=mybir.AxisListType.X, op=ALU.add
            )
            nc.vector.reciprocal(out=r16, in_=zsum16)

            # attn_h[b] = r * attn_psum
            attn_row = small.tile([1, H], f32, tag="attn_row")
            nc.vector.tensor_scalar_mul(attn_row, attn_psum, r16[0:1, :])
            nc.sync.dma_start(out=attn_d[b], in_=attn_row)

            # a (normalized weights), transposed to [16, 128] for contiguous output
            eT_psum = sps.tile([NCH, 128], f32, tag="eT")
            nc.tensor.transpose(eT_psum, e16, ident)
            a_sb = small.tile([NCH, 128], f32, tag="a_sb")
            nc.vector.tensor_scalar_mul(a_sb, eT_psum, r16)
            nc.sync.dma_start(
                out=av_d[b, :, 0].rearrange("(c s) -> c s", c=NCH), in_=a_sb
            )

            # new_coverage = coverage + a
            covT = small.tile([NCH, 128], f32, tag="covT")
            nc.sync.dma_start(
                out=covT, in_=cov_d[b, :, 0].rearrange("(c s) -> c s", c=NCH)
            )
            ncov_sb = small.tile([NCH, 128], f32, tag="ncov_sb")
            nc.vector.tensor_tensor(out=ncov_sb, in0=covT, in1=a_sb, op=ALU.add)
            nc.sync.dma_start(
                out=ncov_d[b, :, 0].rearrange("(c s) -> c s", c=NCH), in_=ncov_sb
            )

    nc.compile()
    return nc


def _get_nc():
    if "nc" not in _CACHE:
        _CACHE["nc"] = _build_nc()
    return _CACHE["nc"]


def _make_in_maps(inputs):
    query = inputs["query"]
    states = inputs["states"]
    states_features = inputs["states_features"]
    coverage = inputs["coverage"]
    Wq = np.ascontiguousarray(inputs["Wq"], dtype=np.float32)
    bq = np.ascontiguousarray(inputs["bq"], dtype=np.float32)
    Wcov = np.ascontiguousarray(inputs["Wcov"], dtype=np.float32)
    v = np.ascontiguousarray(inputs["v"], dtype=np.float32)

    in_maps = []
    for i in range(NCORES):
        sl = slice(i * BPC, (i + 1) * BPC)
        in_maps.append(
            {
                "query": np.ascontiguousarray(query[sl], dtype=np.float32),
                "states": np.ascontiguousarray(states[sl], dtype=np.float32),
                "sfeat": np.ascontiguousarray(states_features[sl], dtype=np.float32),
                "cov": np.ascontiguousarray(coverage[sl], dtype=np.float32),
                "Wq": Wq,
                "bq": bq,
                "Wcov": Wcov,
                "v": v,
            }
        )
    return in_maps


def kernel(query, states, states_features, coverage, source_mask, Wq, bq, Wcov, v):
    from concourse.bass_utils import run_bass_kernel_spmd

    nc = _get_nc()
    in_maps = _make_in_maps(
        {
            "query": query,
            "states": states,
            "states_features": states_features,
            "coverage": coverage,
            "Wq": Wq,
            "bq": bq,
            "Wcov": Wcov,
            "v": v,
        }
    )
    res = run_bass_kernel_spmd(nc, in_maps, core_ids=list(range(NCORES)))
    attn = np.concatenate([r["attn"] for r in res.results], axis=0)
    ncov = np.concatenate([r["ncov"] for r in res.results], axis=0)
    av = np.concatenate([r["av"] for r in res.results], axis=0)
    return attn, ncov, av


# revision 34
# speedup vs baseline: 2.2402x; 1.9997x over previous
"""Trainium2 Bass kernel for coverage-attention (sparse_attention nn_Attention).

Computes, for inputs query(B,1,H) states(B,S,H) states_features(B,S,H)
coverage(B,S,1) source_mask(B,S) and params Wq(H,H) bq(H) Wcov(H,1) v(H):
    qf    = query @ Wq.T + bq
    total = qf + states_features + coverage * Wcov[:,0]
    align = tanh(total) @ v
    a     = softmax(align, axis=1)          (source_mask is all-True)
    attn_h = a @ states
    new_coverage = coverage + a[..., None]
    align_vectors = a[..., None]

Sharding: pure data-parallel over batch, B=32 -> 4 batches on each of 8 cores.
"""

import sys

import numpy as np

if "/opt/trn_rl_repo" not in sys.path:
    sys.path.insert(0, "/opt/trn_rl_repo")

B, S, H = 32, 2048, 1024
NCORES = 8
BPC = B // NCORES  # batches per core
NCH = S // 128     # s-chunks per batch (16)
HC = H // 128      # h-chunks (8)

_CACHE = {}

# Which chunks compute the M-term (qf + cov*Wcov) on DVE instead of PE.
# Every DVE_M_EVERY-th chunk uses the DVE path; balances PE vs DVE load
# (HW-measured: 191us vs 224us for the all-PE variant).
DVE_M_EVERY = 2
ST_ACT_RING = True  # states stream on the ACT HWDGE ring (vs all on SP)


def _bcast_p(ap, p):
    """AP broadcast across p partitions (partition step 0)."""
    import concourse.bass as bass

    return bass.AP(tensor=ap.tensor, offset=ap.offset, ap=[[0, p]] + list(ap.ap))


def _build_nc(repeat=1):
    from contextlib import ExitStack

    import concourse.bacc as bacc
    import concourse.bass as bass
    import concourse.tile as tile
    from concourse import mybir
    from concourse.masks import make_identity

    f32 = mybir.dt.float32
    AF = mybir.ActivationFunctionType
    ALU = mybir.AluOpType

    nc = bacc.Bacc(
        "TRN2", target_bir_lowering=False, debug=False, num_devices=NCORES
    )

    q_d = nc.dram_tensor("query", (BPC, 1, H), f32, kind="ExternalInput").ap()
    st_d = nc.dram_tensor("states", (BPC, S, H), f32, kind="ExternalInput").ap()
    sf_d = nc.dram_tensor("sfeat", (BPC, S, H), f32, kind="ExternalInput").ap()
    cov_d = nc.dram_tensor("cov", (BPC, S, 1), f32, kind="ExternalInput").ap()
    wq_d = nc.dram_tensor("Wq", (H, H), f32, kind="ExternalInput").ap()
    bq_d = nc.dram_tensor("bq", (H,), f32, kind="ExternalInput").ap()
    wcov_d = nc.dram_tensor("Wcov", (H, 1), f32, kind="ExternalInput").ap()
    v_d = nc.dram_tensor("v", (H,), f32, kind="ExternalInput").ap()

    attn_d = nc.dram_tensor("attn", (BPC, 1, H), f32, kind="ExternalOutput").ap()
    ncov_d = nc.dram_tensor("ncov", (BPC, S, 1), f32, kind="ExternalOutput").ap()
    av_d = nc.dram_tensor("av", (BPC, S, 1), f32, kind="ExternalOutput").ap()

    with tile.TileContext(nc) as tc:
      for _rep in range(repeat):
       with ExitStack() as ctx:
        singles = ctx.enter_context(tc.tile_pool(name="singles", bufs=1))
        sf_pool = ctx.enter_context(tc.tile_pool(name="sf", bufs=4))
        st_pool = ctx.enter_context(tc.tile_pool(name="st", bufs=4))
        t_pool = ctx.enter_context(tc.tile_pool(name="t", bufs=3))
        th_pool = ctx.enter_context(tc.tile_pool(name="th", bufs=3))
        pr_pool = ctx.enter_context(tc.tile_pool(name="pr", bufs=2))
        perb = ctx.enter_context(tc.tile_pool(name="perb", bufs=2))
        small = ctx.enter_context(tc.tile_pool(name="small", bufs=2))
        mps = ctx.enter_context(tc.tile_pool(name="mps", bufs=2, space="PSUM"))
        aps_ = ctx.enter_context(tc.tile_pool(name="aps", bufs=1, space="PSUM"))
        sps = ctx.enter_context(tc.tile_pool(name="sps", bufs=1, space="PSUM"))

        # ---- one-time setup ----
        ones1 = singles.tile([1, 128], f32)
        nc.vector.memset(ones1, 1.0)

        def pe_bcast(rep_tile, row_ap):
            for n in range(2):
                bc_ps = mps.tile([128, 512], f32, tag="m_psum", name="bc_ps")
                nc.tensor.matmul(
                    bc_ps,
                    lhsT=ones1,
                    rhs=row_ap[:, bass.ts(n, 512)],
                    start=True,
                    stop=True,
                )
                nc.vector.tensor_copy(
                    out=rep_tile[:, bass.ts(n, 512)], in_=bc_ps
                )

        v_row = singles.tile([1, H], f32)
        nc.sync.dma_start(out=v_row, in_=v_d[None, :])
        v_rep = singles.tile([128, H], f32)
        pe_bcast(v_rep, v_row)

        w_row = singles.tile([1, H], f32)
        nc.sync.dma_start(out=w_row, in_=wcov_d[:, 0][None, :])
        w_rep = singles.tile([128, H], f32)
        pe_bcast(w_rep, w_row)

        dram = ctx.enter_context(tc.tile_pool(name="dram", bufs=1, space="DRAM"))
        qf_scr = dram.tile([BPC, H], f32)

        ident = singles.tile([128, 128], f32)
        make_identity(nc, ident)

        ones128 = singles.tile([128, NCH], f32)
        nc.vector.memset(ones128, 1.0)

        # Wq transposed tiles: wqt[c][p, o] = Wq[o, c*128+p].
        # Load Wq natively (contiguous DMA) and transpose 128x128 blocks on PE;
        # a direct transposed DMA load is ~40x slower (4-byte descriptors).
        wqt = [singles.tile([128, H], f32, tag=f"wqt{c}", name=f"wqt{c}") for c in range(HC)]
        for oc in range(HC):
            wqn = t_pool.tile([128, H], f32, tag="wqn")
            nc.sync.dma_start(out=wqn, in_=wq_d[oc * 128 : (oc + 1) * 128, :])
            for c in range(HC):
                wtp = mps.tile([128, 128], f32, tag="m_psum", name="wtp")
                nc.tensor.transpose(wtp, wqn[:, c * 128 : (c + 1) * 128], ident)
                # DVE copy: this runs in the prologue where DVE is idle and
                # every batch is gated on qf — ACT f32 copies are ~9x slower
                nc.vector.tensor_copy(
                    out=wqt[c][:, oc * 128 : (oc + 1) * 128], in_=wtp
                )

        # qT[p, c, b] = query[b, 0, c*128+p]
        qT = singles.tile([128, HC, BPC], f32)
        for c in range(HC):
            nc.sync.dma_start(
                out=qT[:, c, :],
                in_=q_d[:, 0, c * 128 : (c + 1) * 128].rearrange("b p -> p b"),
            )

        # qf[b, o] = sum_h q[b,h] Wq[o,h]  (+ bq)
        qf_psum = mps.tile([128, H], f32, tag="m_psum")  # only rows 0..BPC used
        for c in range(HC):
            for n in range(2):
                nc.tensor.matmul(
                    qf_psum[:BPC, bass.ts(n, 512)],
                    lhsT=qT[:, c, :],
                    rhs=wqt[c][:, bass.ts(n, 512)],
                    start=(c == 0),
                    stop=(c == HC - 1),
                )
        bq4 = small.tile([BPC, H], f32, tag="bq4")
        nc.sync.dma_start(out=bq4, in_=_bcast_p(bq_d, BPC))
        qf_sb = singles.tile([BPC, H], f32)
        nc.vector.tensor_tensor(
            out=qf_sb, in0=qf_psum[:BPC, :], in1=bq4, op=ALU.add
        )
        # round-trip through DRAM so qf can be partition-broadcast per batch
        nc.sync.dma_start(out=qf_scr, in_=qf_sb)

        # ---- per-batch streaming ----
        for b in range(BPC):
            # rhs2: row0 = qf[b,:], row1 = Wcov[:,0]
            rhs2 = perb.tile([2, H], f32, tag="rhs2")
            nc.sync.dma_start(out=rhs2[0:1, :], in_=qf_sb[b : b + 1, :])
            nc.sync.dma_start(out=rhs2[1:2, :], in_=wcov_d[:, 0][None, :])

            # lhs2: row0 = ones, row1 = coverage[b,:,0]
            lhs2 = perb.tile([2, S], f32, tag="lhs2")
            nc.vector.memset(lhs2[0:1, :], 1.0)
            nc.sync.dma_start(out=lhs2[1:2, :], in_=cov_d[b, :, 0][None, :])

            # qf[b] replicated across 128 partitions (for the DVE M-path)
            qf_row = small.tile([1, H], f32, tag="qf_row")
            nc.sync.dma_start(out=qf_row, in_=qf_scr[b, :][None, :])
            qf_rep = perb.tile([128, H], f32, tag="qf_rep")
            pe_bcast(qf_rep, qf_row)

            e16 = perb.tile([128, NCH], f32, tag="e16")
            align16 = perb.tile([128, NCH], f32, tag="al16")
            attn_psum = aps_.tile([1, H], f32)

            for c in range(NCH):
                sf_t = sf_pool.tile([128, H], f32)
                nc.sync.dma_start(out=sf_t, in_=sf_d[b, bass.ts(c, 128), :])
                st_t = st_pool.tile([128, H], f32)
                # states stream on the ACT HWDGE ring, sfeat on the SP ring —
                # two hardware DGE FIFOs issue in parallel
                st_dma_eng = nc.scalar if ST_ACT_RING else nc.sync
                st_dma_eng.dma_start(out=st_t, in_=st_d[b, bass.ts(c, 128), :])

                t_t = t_pool.tile([128, H], f32)
                if DVE_M_EVERY == 0 or c % DVE_M_EVERY != DVE_M_EVERY - 1:
                    # PE path: M[s,h] = qf[h] + cov[s]*Wcov[h] (rank-2, K=2)
                    m_psum = mps.tile([128, H], f32)
                    for n in range(2):
                        nc.tensor.matmul(
                            m_psum[:, bass.ts(n, 512)],
                            lhsT=lhs2[:, bass.ts(c, 128)],
                            rhs=rhs2[:, bass.ts(n, 512)],
                            start=True,
                            stop=True,
                        )
                    nc.vector.tensor_tensor(
                        out=t_t, in0=sf_t, in1=m_psum, op=ALU.add
                    )
                else:
                    # DVE path: (Wcov_rep * cov[s]) + sf, then + qf_rep
                    covc = small.tile([128, 1], f32, tag="covc")
                    nc.sync.dma_start(
                        out=covc, in_=cov_d[b, bass.ts(c, 128), :]
                    )
                    msf = pr_pool.tile([128, H], f32, tag="msf")
                    nc.vector.scalar_tensor_tensor(
                        out=msf,
                        in0=w_rep,
                        scalar=covc,
                        in1=sf_t,
                        op0=ALU.mult,
                        op1=ALU.add,
                    )
                    nc.vector.tensor_tensor(
                        out=t_t, in0=msf, in1=qf_rep, op=ALU.add
                    )
                th_t = th_pool.tile([128, H], f32)
                nc.scalar.activation(out=th_t, in_=t_t, func=AF.Tanh)
                prod = pr_pool.tile([128, H], f32)
                nc.vector.scalar_tensor_tensor(
                    out=prod,
                    in0=th_t,
                    scalar=1.0,
                    in1=v_rep,
                    op0=ALU.mult,
                    op1=ALU.mult,
                    accum_out=align16[:, c : c + 1],
                )
                nc.scalar.activation(
                    out=e16[:, c : c + 1],
                    in_=align16[:, c : c + 1],
                    func=AF.Exp,
                )
                for n in range(2):
                    nc.tensor.matmul(
                        attn_psum[:, bass.ts(n, 512)],
                        lhsT=e16[:, c : c + 1],
                        rhs=st_t[:, bass.ts(n, 512)],
                        start=(c == 0),
                        stop=(c == NCH - 1),
                    )

            # Z replicated on 16 partitions: z16[m, c] = sum_p e16[p, c] for all m
            z_psum = sps.tile([NCH, NCH], f32, tag="z")
            nc.tensor.matmul(z_psum, lhsT=ones128, rhs=e16, start=True, stop=True)
            r16 = small.tile([NCH, 1], f32, tag="r16")
            zsum16 = small.tile([NCH, 1], f32, tag="zsum16")
            nc.vector.tensor_reduce(
                out=zsum16, in_=z_psum, axis=mybir.AxisListType.X, op=ALU.add
            )
            nc.vector.reciprocal(out=r16, in_=zsum16)

            # attn_h[b] = r * attn_psum
            attn_row = small.tile([1, H], f32, tag="attn_row")
            nc.vector.tensor_scalar_mul(attn_row, attn_psum, r16[0:1, :])
            nc.sync.dma_start(out=attn_d[b], in_=attn_row)

            # a (normalized weights), transposed to [16, 128] for contiguous output
            eT_psum = sps.tile([NCH, 128], f32, tag="eT")
            nc.tensor.transpose(eT_psum, e16, ident)
            a_sb = small.tile([NCH, 128], f32, tag="a_sb")
            nc.vector.tensor_scalar_mul(a_sb, eT_psum, r16)
            nc.sync.dma_start(
                out=av_d[b, :, 0].rearrange("(c s) -> c s", c=NCH), in_=a_sb
            )

            # new_coverage = coverage + a
            covT = small.tile([NCH, 128], f32, tag="covT")
            nc.sync.dma_start(
                out=covT, in_=cov_d[b, :, 0].rearrange("(c s) -> c s", c=NCH)
            )
            ncov_sb = small.tile([NCH, 128], f32, tag="ncov_sb")
            nc.vector.tensor_tensor(out=ncov_sb, in0=covT, in1=a_sb, op=ALU.add)
            nc.sync.dma_start(
                out=ncov_d[b, :, 0].rearrange("(c s) -> c s", c=NCH), in_=ncov_sb
            )

    nc.compile()
    return nc


def _get_nc():
    if "nc" not in _CACHE:
        _CACHE["nc"] = _build_nc()
    return _CACHE["nc"]


def _make_in_maps(inputs):
    query = inputs["query"]
    states = inputs["states"]
    states_features = inputs["states_features"]
    coverage = inputs["coverage"]
    Wq = np.ascontiguousarray(inputs["Wq"], dtype=np.float32)
    bq = np.ascontiguousarray(inputs["bq"], dtype=np.float32)
    Wcov = np.ascontiguousarray(inputs["Wcov"], dtype=np.float32)
    v = np.ascontiguousarray(inputs["v"], dtype=np.float32)

    in_maps = []
    for i in range(NCORES):
        sl = slice(i * BPC, (i + 1) * BPC)
        in_maps.append(
            {
                "query": np.ascontiguousarray(query[sl], dtype=np.float32),
                "states": np.ascontiguousarray(states[sl], dtype=np.float32),
                "sfeat": np.ascontiguousarray(states_features[sl], dtype=np.float32),
                "cov": np.ascontiguousarray(coverage[sl], dtype=np.float32),
                "Wq": Wq,
                "bq": bq,
                "Wcov": Wcov,
                "v": v,
            }
        )
    return in_maps


def kernel(query, states, states_features, coverage, source_mask, Wq, bq, Wcov, v):
    from concourse.bass_utils import run_bass_kernel_spmd

    nc = _get_nc()
    in_maps = _make_in_maps(
        {
            "query": query,
            "states": states,
            "states_features": states_features,
            "coverage": coverage,
            "Wq": Wq,
            "bq": bq,
            "Wcov": Wcov,
            "v": v,
        }
    )
    res = run_bass_kernel_spmd(nc, in_maps, core_ids=list(range(NCORES)))
    attn = np.concatenate([r["attn"] for r in res.results], axis=0)
    ncov = np.concatenate([r["ncov"] for r in res.results], axis=0)
    av = np.concatenate([r["av"] for r in res.results], axis=0)
    return attn, ncov, av
